# revision 13
# baseline (speedup 1.0000x reference)
"""DeBERTa-RoPE self-attention on 8 Trainium2 cores.

Sharding: data-parallel over batch (4) x tensor-parallel over heads (2 groups
of 8). Each core computes the qkv projection for its (batch, head-group),
RoPE, attention, and a row-parallel partial out-projection; the host sums the
two partials per batch and applies the (constant) v/out bias correction.

Device schedule (per core), engine-balanced and software-pipelined:
 - head: input DMAs spread over SP/Pool/DVE/ACT queues; v-projection
   (kt-outer, 8 psum banks); qk projection + RoPE for head-pair 0.
 - windows p=0..3: scores (bf16 matmuls) -> exp (ACT, the window pacer)
   -> transposed context accumulation (out[q,d] += ex^T v, 64-row matmuls)
   plus 1-row denominator matmuls, while the qk projection + RoPE chain for
   head-pair p+1 interleaves into the same PE stream.
 - tail: PE transposes of the normalized context back to [d, q], partial
   out-projection, yT DMA.

RoPE's rotate-half is a partition-block shuffle done with SBUF->SBUF DMAs
(sign folded into sinT); softmax skips max-subtraction (|scores| <= ~5);
masking is folded into v and the denominator column.
"""

import numpy as np
import ml_dtypes

import concourse.bass as bass
import concourse.mybir as mybir
import concourse.tile as tile
from concourse.bass_utils import run_bass_kernel_spmd

H = 16
D = 64
HID = H * D
B = 4
S = 1024
THETA = 10000.0
NCORES = 8
HPC = H // 2          # heads per core
KT = HID // 128       # 8 k-tiles
ST = S // 128         # 8 seq tiles
LAG = 3               # ctx trails scores by LAG tt-slots

F32 = mybir.dt.float32
BF16 = mybir.dt.bfloat16
AF = mybir.ActivationFunctionType
ALU = mybir.AluOpType


def build_program():
    nc = bass.Bass()
    xT = nc.declare_dram_parameter("xT", [HID, S], BF16, isOutput=False)
    wqk = nc.declare_dram_parameter("wqk", [HID, 1024], BF16, isOutput=False)
    wv = nc.declare_dram_parameter("wv", [HID, 512], BF16, isOutput=False)
    wout = nc.declare_dram_parameter("wout", [512, HID], BF16, isOutput=False)
    bqk = nc.declare_dram_parameter("bqk", [128, 8], F32, isOutput=False)
    cosT = nc.declare_dram_parameter("cosT", [128, S], BF16, isOutput=False)
    sinT = nc.declare_dram_parameter("sinT", [128, S], BF16, isOutput=False)
    permT = nc.declare_dram_parameter("permT", [128, 128], BF16,
                                      isOutput=False)
    mcol = nc.declare_dram_parameter("mcol", [128, ST], F32, isOutput=False)
    mcolB = nc.declare_dram_parameter("mcolB", [128, ST], BF16, isOutput=False)
    ident = nc.declare_dram_parameter("ident", [128, 128], BF16, isOutput=False)
    yT = nc.declare_dram_parameter("yT", [HID, S], F32, isOutput=True)

    with tile.TileContext(nc) as tc:
        with (
            tc.tile_pool(name="const", bufs=1) as cpool,
            tc.tile_pool(name="persist", bufs=1) as persist,
        ):
            cos_sb = cpool.tile([128, S], BF16)
            sin_sb = cpool.tile([128, S], BF16)
            mcol_sb = cpool.tile([128, ST], F32)
            mcolB_sb = cpool.tile([128, ST], BF16)
            bqk_sb = cpool.tile([128, 8], F32)
            permT_sb = cpool.tile([128, 128], BF16)
            ident_sb = cpool.tile([128, 128], BF16)

            xT_sb = persist.tile([128, KT, S], BF16)
            wqk_sb = persist.tile([128, KT, 1024], BF16)
            wv_sb = persist.tile([128, KT, 512], BF16)
            wout_sb = persist.tile([128, 4, HID], BF16)
            rope_sb = persist.tile([128, 8, S], BF16)
            vmask_sb = persist.tile([128, ST, HPC, 64], BF16)
            ctxq_sb = persist.tile([128, 4, ST, 128], BF16)
            ctxT_sb = persist.tile([128, 4, S], BF16)

            # ---- input DMAs, spread across engine queues ----
            for kt in range(KT):
                nc.sync.dma_start(
                    xT_sb[:, kt, :], xT[kt * 128:(kt + 1) * 128, :])
                nc.gpsimd.dma_start(
                    wqk_sb[:, kt, :], wqk[kt * 128:(kt + 1) * 128, :])
                nc.scalar.dma_start(
                    wv_sb[:, kt, :], wv[kt * 128:(kt + 1) * 128, :])
            nc.scalar.dma_start(cos_sb[:], cosT[:])
            nc.scalar.dma_start(sin_sb[:], sinT[:])
            nc.scalar.dma_start(mcol_sb[:], mcol[:])
            nc.scalar.dma_start(mcolB_sb[:], mcolB[:])
            nc.scalar.dma_start(bqk_sb[:], bqk[:])
            nc.scalar.dma_start(permT_sb[:], permT[:])
            nc.scalar.dma_start(ident_sb[:], ident[:])

            # ---- attention-era pools (psum: qk 1 + sc 4 + ctx 2 + den 1) ----
            with (
                tc.tile_pool(name="psQk", bufs=1, space="PSUM") as psQk,
                tc.tile_pool(name="rt", bufs=2) as rt,
                tc.tile_pool(name="expool", bufs=8) as expool,
                tc.tile_pool(name="small", bufs=2) as small,
            ):
                # qk projection chunk: 8 accumulating matmuls into one bank
                def emit_qk_chunk_mms(m, ch):
                    ps = psQk.tile([128, 512], F32, tag="qk",
                                   name=f"qk{m}_{ch}")
                    for kt in range(KT):
                        nc.tensor.matmul(
                            ps[:],
                            wqk_sb[:, kt, m * 128:(m + 1) * 128],
                            xT_sb[:, kt, ch * 512:(ch + 1) * 512],
                            start=(kt == 0), stop=(kt == KT - 1),
                        )
                    return ps

                # RoPE part 1 (right after the qk matmuls): bias-add copy to
                # SBUF and the cos-term; both feed part 2.
                def emit_rope_a(m, ch, ps):
                    sl = slice(ch * 512, (ch + 1) * 512)
                    qkb = rt.tile([128, 512], BF16, tag="qkb",
                                  name=f"qkb{m}_{ch}")
                    nc.vector.tensor_scalar_add(
                        qkb[:], ps[:], bqk_sb[:, m:m + 1])
                    t1 = rt.tile([128, 512], BF16, tag="t1")
                    nc.vector.tensor_mul(t1[:], qkb[:], cos_sb[:, sl])
                    return qkb, t1

                # RoPE part 2: rotate-half via perm matmul (reuses the qk
                # bank; bqksh folds in since bqksh = P @ bqk), sin-term, sum.
                def emit_rope_b(m, ch, qkb, t1):
                    sl = slice(ch * 512, (ch + 1) * 512)
                    ps_sh = psQk.tile([128, 512], F32, tag="qk",
                                      name=f"sh{m}_{ch}")
                    nc.tensor.matmul(ps_sh[:], permT_sb[:], qkb[:],
                                     start=True, stop=True)
                    s2 = rt.tile([128, 512], BF16, tag="s2")
                    nc.vector.tensor_mul(s2[:], ps_sh[:], sin_sb[:, sl])
                    nc.vector.tensor_add(rope_sb[:, m, sl], t1[:], s2[:])

                # chunk order per p: k-ch0, q-ch0, q-ch1, k-ch1 (matches the
                # order the consumer scores matmuls need them)
                def chunk_list(p):
                    return [(p + 4, 0), (p, 0), (p, 1), (p + 4, 1)]

                # ---- head: v projection (2 epochs of 4 banks) interleaved
                # with the p=0 qk projection + RoPE ----
                with tc.tile_pool(name="psV", bufs=1, space="PSUM") as psV:
                    def v_epoch(ep):
                        tts = range(ep * 4, ep * 4 + 4)
                        vps = {tt: psV.tile([128, 512], F32, tag=f"v{tt % 4}",
                                            name=f"vps{tt}") for tt in tts}
                        for kt in range(KT):
                            for tt in tts:
                                nc.tensor.matmul(
                                    vps[tt][:],
                                    xT_sb[:, kt, tt * 128:(tt + 1) * 128],
                                    wv_sb[:, kt, :],
                                    start=(kt == 0), stop=(kt == KT - 1),
                                )
                        for tt in tts:
                            nc.vector.tensor_scalar_mul(
                                vmask_sb[:, tt, :, :]
                                .rearrange("p h d -> p (h d)"),
                                vps[tt][:], mcol_sb[:, tt:tt + 1])

                    cl0 = chunk_list(0)
                    v_epoch(0)
                    st0 = []
                    for (m, ch) in cl0[:2]:
                        ps = emit_qk_chunk_mms(m, ch)
                        st0.append(emit_rope_a(m, ch, ps))
                        emit_rope_b(m, ch, *st0[-1])
                    v_epoch(1)
                    for (m, ch) in cl0[2:]:
                        ps = emit_qk_chunk_mms(m, ch)
                        st0.append(emit_rope_a(m, ch, ps))
                        emit_rope_b(m, ch, *st0[-1])

                # ---- windows ----
                with (
                    tc.tile_pool(name="psSc", bufs=1, space="PSUM") as psSc,
                    tc.tile_pool(name="psCtx", bufs=1, space="PSUM") as psCtx,
                    tc.tile_pool(name="psDen", bufs=1, space="PSUM") as psDen,
                ):
                    # scores + exp for one (p, tt, ch)
                    def emit_scores(p, tt, ch, seq):
                        qp = rope_sb[:, p, :]
                        kp = rope_sb[:, p + 4, :]
                        ps = psSc.tile([128, 2, 512], F32, tag=f"sc{seq % 2}",
                                       name=f"sc{p}_{tt}_{ch}")
                        for hh in range(2):
                            base = hh * 64
                            nc.tensor.matmul(
                                ps[:, hh, :],
                                kp[base:base + 64, tt * 128:(tt + 1) * 128],
                                qp[base:base + 64, ch * 512:(ch + 1) * 512],
                                start=True, stop=True,
                                tile_position=(base, 0),
                            )
                        ex = expool.tile([128, 2, 512], BF16, tag="ex",
                                         name=f"ex{p}_{tt}_{ch}")
                        nc.scalar.activation(ex[:], ps[:], AF.Exp,
                                             scale=0.125)
                        return ex

                    # transposed ctx + denominator for one (p, tt)
                    def emit_ctx_den(p, tt, exs, ctx_ps, den_ps):
                        for ch in range(2):
                            ex = exs[ch]
                            for hh in range(2):
                                for jc in range(4):
                                    jj = ch * 4 + jc
                                    lhsT = ex[:, hh, jc * 128:(jc + 1) * 128]
                                    first = (tt == 0 and ch == 0 and jc == 0)
                                    last = (tt == ST - 1 and ch == 1
                                            and jc == 3)
                                    nc.tensor.matmul(
                                        ctx_ps[hh][:, jj, :],
                                        lhsT,
                                        vmask_sb[:, tt, 2 * p + hh, :],
                                        start=first, stop=last,
                                    )
                                    dfirst = (tt == 0 and ch == 0
                                              and hh == 0 and jc == 0)
                                    dlast = (tt == ST - 1 and ch == 1
                                             and hh == 1 and jc == 3)
                                    nc.tensor.matmul(
                                        den_ps[:, hh * 8 + jj:
                                               hh * 8 + jj + 1],
                                        lhsT,
                                        mcolB_sb[:, tt:tt + 1],
                                        start=dfirst, stop=dlast,
                                    )

                    def emit_ctx_norm(p, ctx_ps, den_ps):
                        recip = small.tile([128, 16], F32, tag="recip")
                        nc.vector.reciprocal(recip[:], den_ps[:, 0:16])
                        for hh in range(2):
                            for jj in range(ST):
                                nc.vector.tensor_scalar_mul(
                                    ctxq_sb[:, p, jj, hh * 64:(hh + 1) * 64],
                                    ctx_ps[hh][:, jj, :],
                                    recip[:, hh * 8 + jj:hh * 8 + jj + 1])

                    seq = 0
                    for p in range(4):
                        ctx_ps = [psCtx.tile([128, ST, 64], F32,
                                             tag=f"ctx{hh}",
                                             name=f"ctx{p}_{hh}")
                                  for hh in range(2)]
                        den_ps = psDen.tile([128, 512], F32, tag="den",
                                            name=f"den{p}")
                        exs_by_tt = {}
                        nxt = chunk_list(p + 1) if p < 3 else []
                        st = None
                        for tt in range(ST):
                            exs = []
                            for ch in range(2):
                                exs.append(emit_scores(p, tt, ch, seq))
                                seq += 1
                            exs_by_tt[tt] = exs
                            if tt >= LAG:
                                emit_ctx_den(p, tt - LAG,
                                             exs_by_tt.pop(tt - LAG),
                                             ctx_ps, den_ps)
                            # interleave next head-pair's qk + rope chain:
                            # even slot: qk matmuls + bias/cos terms,
                            # odd slot: perm matmul + sin term + sum
                            if nxt:
                                m, ch = nxt[tt // 2]
                                if tt % 2 == 0:
                                    ps = emit_qk_chunk_mms(m, ch)
                                    st = emit_rope_a(m, ch, ps)
                                else:
                                    emit_rope_b(m, ch, *st)
                            if p == 2 and tt == 0:
                                for kt in range(4):
                                    nc.sync.dma_start(
                                        wout_sb[:, kt, :],
                                        wout[kt * 128:(kt + 1) * 128, :])
                        for tt in range(ST - LAG, ST):
                            emit_ctx_den(p, tt, exs_by_tt.pop(tt),
                                         ctx_ps, den_ps)
                        emit_ctx_norm(p, ctx_ps, den_ps)

            # ---- tail: transpose ctx back to [d, q], out-projection ----
            with (
                tc.tile_pool(name="psT", bufs=2, space="PSUM") as psT,
                tc.tile_pool(name="psY", bufs=1, space="PSUM") as psY,
                tc.tile_pool(name="ytp", bufs=3) as ytp,
            ):
                for p in range(4):
                    for half in range(2):
                        pst = psT.tile([128, 4, 256], BF16, tag="t")
                        for q in range(4):
                            nc.tensor.matmul(
                                pst[:, q, 0:128],
                                ctxq_sb[:, p, half * 4 + q, :],
                                ident_sb[:],
                                is_transpose=True, start=True, stop=True)
                        nc.vector.tensor_copy(
                            ctxT_sb[:, p, half * 512:(half + 1) * 512]
                            .rearrange("p (q f) -> p q f", f=128),
                            pst[:, :, 0:128])
                for m in range(8):
                    psy = psY.tile([128, 1024], F32, tag=f"y{m % 2}",
                                   name=f"psy{m}")
                    for ch in range(2):
                        for kt in range(4):
                            nc.tensor.matmul(
                                psy[:, ch * 512:(ch + 1) * 512],
                                wout_sb[:, kt, m * 128:(m + 1) * 128],
                                ctxT_sb[:, kt, ch * 512:(ch + 1) * 512],
                                start=(kt == 0), stop=(kt == 3))
                    yt = ytp.tile([128, 1024], F32, tag="yt", name=f"yt{m}")
                    if m % 2 == 0:
                        nc.scalar.copy(yt[:], psy[:])
                    else:
                        nc.vector.tensor_copy(yt[:], psy[:])
                    deng = (nc.sync, nc.gpsimd, nc.scalar)[m % 3]
                    deng.dma_start(yT[m * 128:(m + 1) * 128, :], yt[:])

    return nc


def _split_waits(nc, max_waits=1):
    """This walrus build rejects >1 sync-wait command per instruction; hoist
    extra waits onto preceding NoOps on the same engine/queue."""
    for bb in nc.main_func.blocks:
        new_insts = []
        for ins in bb.instructions:
            si = getattr(ins, "sync_info", None)
            if si is not None and si.on_wait and len(si.on_wait) > max_waits:
                waits = list(si.on_wait)
                head, rest = waits[:max_waits], waits[max_waits:]
                while rest:
                    chunk, rest = rest[:max_waits], rest[max_waits:]
                    new_insts.append(mybir.InstNoOp(
                        name=f"waitsplit-{nc.next_id()}", ins=[], outs=[],
                        sync_info=mybir.SyncInfo(on_wait=chunk, on_update=[]),
                        engine=ins.engine))
                ins.sync_info = mybir.SyncInfo(
                    on_wait=head, on_update=list(si.on_update or []))
            new_insts.append(ins)
        bb.instructions = new_insts


def make_core_inputs(x, attention_mask, Wqkv, bqkv, Wout):
    """Host-side shard prep: returns list of 8 in_maps (core = 2*b + g)."""
    BF = ml_dtypes.bfloat16
    Wr = np.ascontiguousarray(Wqkv).reshape(HID, 3, H, D)
    br = np.ascontiguousarray(bqkv).reshape(3, H, D)

    inv = 1.0 / (THETA ** (np.arange(0, D, 2, dtype=np.float64) / D))
    pos = np.arange(S, dtype=np.float64)
    freqs = pos[:, None] * inv[None, :]              # [S, 32]
    emb = np.concatenate([freqs, freqs], axis=1)     # [S, 64]
    cosT = np.cos(emb).T.astype(np.float32)          # [64, S]
    sgn = np.concatenate([-np.ones(32), np.ones(32)])[:, None]
    sinTs = (sgn * np.sin(emb).T).astype(np.float32)
    cos2 = np.concatenate([cosT, cosT], 0).astype(BF)   # [128, S]
    sin2 = np.concatenate([sinTs, sinTs], 0).astype(BF)
    ident = np.eye(128, dtype=np.float32).astype(BF)
    pp = np.arange(128)
    shmap = (pp - pp % 64) + (pp % 64 + 32) % 64
    permTm = np.zeros((128, 128), dtype=np.float32)
    permTm[shmap, pp] = 1.0
    permTm = permTm.astype(BF)

    in_maps = []
    for c in range(NCORES):
        b, g = c // 2, c % 2
        hs = slice(g * HPC, (g + 1) * HPC)
        wqk = np.concatenate(
            [Wr[:, 0, hs, :].reshape(HID, 512),
             Wr[:, 1, hs, :].reshape(HID, 512)], axis=1)
        wv = Wr[:, 2, hs, :].reshape(HID, 512)
        bqk = np.concatenate(
            [br[0, hs].reshape(512), br[1, hs].reshape(512)]
        ).reshape(8, 128).T
        mcolv = attention_mask[b].astype(np.float32).reshape(ST, 128).T
        in_maps.append({
            "xT": np.ascontiguousarray(x[b].T.astype(BF)),
            "wqk": np.ascontiguousarray(wqk.astype(BF)),
            "wv": np.ascontiguousarray(wv.astype(BF)),
            "wout": np.ascontiguousarray(
                Wout[g * 512:(g + 1) * 512, :].astype(BF)),
            "bqk": np.ascontiguousarray(bqk.astype(np.float32)),
            "cosT": cos2, "sinT": sin2, "permT": permTm,
            "mcol": np.ascontiguousarray(mcolv),
            "mcolB": np.ascontiguousarray(mcolv.astype(BF)),
            "ident": ident,
        })
    return in_maps


_PROGRAM = None


def kernel(x, attention_mask, Wqkv, bqkv, Wout, bout, _trace=False):
    global _PROGRAM
    x = np.asarray(x)
    attention_mask = np.asarray(attention_mask)
    Wqkv = np.asarray(Wqkv)
    bqkv = np.asarray(bqkv)
    Wout = np.asarray(Wout)
    bout = np.asarray(bout)

    if _PROGRAM is None:
        _PROGRAM = build_program()
        _split_waits(_PROGRAM)
    nc = _PROGRAM

    in_maps = make_core_inputs(x, attention_mask, Wqkv, bqkv, Wout)
    res = run_bass_kernel_spmd(
        nc, in_maps, core_ids=list(range(NCORES)), trace=_trace)

    y = np.empty((B, S, HID), dtype=np.float32)
    for b in range(B):
        acc = res.results[2 * b]["yT"] + res.results[2 * b + 1]["yT"]
        y[b] = acc.T
    # exact host-side bias corrections: v-bias shifts context by a constant
    # (attn rows sum to 1), q/k biases were applied on device.
    bv = bqkv[2 * HID:3 * HID].astype(np.float32)
    y += (bv @ Wout + bout).astype(np.float32)[None, None, :]
    if _trace:
        kernel.last_exec_time_ns = res.exec_time_ns
    return y


# revision 63
# speedup vs baseline: 1.0482x; 1.0482x over previous
"""DeBERTa-RoPE self-attention on 8 Trainium2 cores.

Sharding: data-parallel over batch (4) x tensor-parallel over heads (2 groups
of 8). Each core computes the qkv projection for its (batch, head-group),
RoPE, attention, and a row-parallel partial out-projection; the host sums the
two partials per batch and applies the (constant) v/out bias correction.

Device schedule (per core), engine-balanced and software-pipelined:
 - head: input DMAs spread over SP/Pool/DVE/ACT queues; v-projection
   (kt-outer, 8 psum banks); qk projection + RoPE for head-pair 0.
 - windows p=0..3: scores (bf16 matmuls) -> exp (ACT, the window pacer)
   -> transposed context accumulation (out[q,d] += ex^T v, 64-row matmuls)
   plus 1-row denominator matmuls, while the qk projection + RoPE chain for
   head-pair p+1 interleaves into the same PE stream.
 - tail: PE transposes of the normalized context back to [d, q], partial
   out-projection, yT DMA.

RoPE's rotate-half is a partition-block shuffle done with SBUF->SBUF DMAs
(sign folded into sinT); softmax skips max-subtraction (|scores| <= ~5);
masking is folded into v and the denominator column.
"""

import numpy as np
import ml_dtypes

import concourse.bass as bass
import concourse.mybir as mybir
import concourse.tile as tile
from concourse.bass_utils import run_bass_kernel_spmd

H = 16
D = 64
HID = H * D
B = 4
S = 1024
THETA = 10000.0
NCORES = 8
HPC = H // 2          # heads per core
KT = HID // 128       # 8 k-tiles
ST = S // 128         # 8 seq tiles
LAG = 4               # ctx trails scores by LAG tt-slots

F32 = mybir.dt.float32
BF16 = mybir.dt.bfloat16
F8 = mybir.dt.float8e4
AF = mybir.ActivationFunctionType
ALU = mybir.AluOpType
DR = mybir.MatmulPerfMode.DoubleRow
QSC = 8.0            # wqk pre-scale (keeps fp8 weights out of subnormals)


def build_program():
    nc = bass.Bass()
    xT = nc.declare_dram_parameter("xT", [HID, S], BF16, isOutput=False)
    wqk = nc.declare_dram_parameter("wqk", [HID, 1024], BF16, isOutput=False)
    wv = nc.declare_dram_parameter("wv", [HID, 512], BF16, isOutput=False)
    wout = nc.declare_dram_parameter("wout", [512, HID], BF16, isOutput=False)
    bqk = nc.declare_dram_parameter("bqk", [128, 8], F32, isOutput=False)
    cosT = nc.declare_dram_parameter("cosT", [128, S], BF16, isOutput=False)
    sinT = nc.declare_dram_parameter("sinT", [128, S], BF16, isOutput=False)
    permT = nc.declare_dram_parameter("permT", [128, 128], BF16,
                                      isOutput=False)
    mcol = nc.declare_dram_parameter("mcol", [128, ST], F32, isOutput=False)
    mcolB = nc.declare_dram_parameter("mcolB", [128, ST], BF16, isOutput=False)
    ident = nc.declare_dram_parameter("ident", [128, 128], BF16, isOutput=False)
    yT = nc.declare_dram_parameter("yT", [HID, S], BF16, isOutput=True)

    with tile.TileContext(nc) as tc:
        with (
            tc.tile_pool(name="const", bufs=1) as cpool,
            tc.tile_pool(name="persist", bufs=1) as persist,
        ):
            cos_sb = cpool.tile([128, S], BF16)
            sin_sb = cpool.tile([128, S], BF16)
            mcol_sb = cpool.tile([128, ST], F32)
            mcolB_sb = cpool.tile([128, ST], BF16)
            bqk_sb = cpool.tile([128, 8], F32)
            permT_sb = cpool.tile([128, 128], BF16)
            ident_sb = cpool.tile([128, 128], BF16)

            xT_sb = persist.tile([128, KT, S], BF16)
            wqk_sb = persist.tile([128, KT, 1024], BF16)
            wv_sb = persist.tile([128, KT, 512], BF16)
            wout_sb = persist.tile([128, 4, HID], BF16)
            rope_sb = persist.tile([128, 8, S], BF16)
            vmask_sb = persist.tile([128, ST, HPC, 64], BF16)
            ctxq_sb = persist.tile([128, 4, ST, 128], BF16)
            ctxT_sb = persist.tile([128, 4, S], BF16)

            # ---- input DMAs, spread across engine queues ----
            for kt in range(KT):
                nc.sync.dma_start(
                    xT_sb[:, kt, :], xT[kt * 128:(kt + 1) * 128, :])
                nc.gpsimd.dma_start(
                    wqk_sb[:, kt, :], wqk[kt * 128:(kt + 1) * 128, :])
                nc.scalar.dma_start(
                    wv_sb[:, kt, :], wv[kt * 128:(kt + 1) * 128, :])
            nc.scalar.dma_start(cos_sb[:], cosT[:])
            nc.scalar.dma_start(sin_sb[:], sinT[:])
            nc.scalar.dma_start(mcol_sb[:], mcol[:])
            nc.scalar.dma_start(mcolB_sb[:], mcolB[:])
            nc.scalar.dma_start(bqk_sb[:], bqk[:])
            nc.scalar.dma_start(permT_sb[:], permT[:])
            nc.scalar.dma_start(ident_sb[:], ident[:])

            # ---- attention-era pools (psum: qk 1 + sc 4 + ctx 2 + den 1) ----
            with (
                tc.tile_pool(name="psQk", bufs=1, space="PSUM") as psQk,
                tc.tile_pool(name="rt", bufs=2) as rt,
                tc.tile_pool(name="expool", bufs=10) as expool,
                tc.tile_pool(name="small", bufs=2) as small,
            ):
                # qk projection chunk: 8 accumulating matmuls into one bank
                def emit_qk_chunk_mms(m, ch):
                    ps = psQk.tile([128, 512], F32, tag="qk",
                                   name=f"qk{m}_{ch}")
                    for kt in range(KT):
                        nc.tensor.matmul(
                            ps[:],
                            wqk_sb[:, kt, m * 128:(m + 1) * 128],
                            xT_sb[:, kt, ch * 512:(ch + 1) * 512],
                            start=(kt == 0), stop=(kt == KT - 1),
                        )
                    return ps

                # RoPE part 1 (right after the qk matmuls): bias-add copy to
                # SBUF and the cos-term; both feed part 2.
                def emit_rope_a(m, ch, ps):
                    sl = slice(ch * 512, (ch + 1) * 512)
                    qkb = rt.tile([128, 512], BF16, tag="qkb",
                                  name=f"qkb{m}_{ch}")
                    nc.vector.tensor_scalar_add(
                        qkb[:], ps[:], bqk_sb[:, m:m + 1])
                    t1 = rt.tile([128, 512], BF16, tag="t1")
                    nc.vector.tensor_mul(t1[:], qkb[:], cos_sb[:, sl])
                    return qkb, t1

                # RoPE part 2: rotate-half via perm matmul (reuses the qk
                # bank; bqksh folds in since bqksh = P @ bqk), sin-term, sum
                # (fp8 out), then the pair-shuffle DMA into DoubleRow layout:
                # dims (2a, 2a+1) land in (partition a, slots 0/1).
                def emit_rope_b(m, ch, qkb, t1):
                    sl = slice(ch * 512, (ch + 1) * 512)
                    ps_sh = psQk.tile([128, 512], F32, tag="qk",
                                      name=f"sh{m}_{ch}")
                    nc.tensor.matmul(ps_sh[:], permT_sb[:], qkb[:],
                                     start=True, stop=True)
                    s2 = rt.tile([128, 512], BF16, tag="s2")
                    nc.vector.tensor_mul(s2[:], ps_sh[:], sin_sb[:, sl])
                    nc.vector.tensor_add(rope_sb[:, m, sl], t1[:], s2[:])

                # chunk order per p: k-ch0, q-ch0, q-ch1, k-ch1 (matches the
                # order the consumer scores matmuls need them)
                def chunk_list(p):
                    return [(p + 4, 0), (p, 0), (p, 1), (p + 4, 1)]

                # ---- head: v projection (2 epochs of 4 banks) interleaved
                # with the p=0 qk projection + RoPE ----
                with tc.tile_pool(name="psV", bufs=1, space="PSUM") as psV:
                    def v_epoch(ep):
                        tts = range(ep * 4, ep * 4 + 4)
                        vps = {tt: psV.tile([128, 512], F32, tag=f"v{tt % 4}",
                                            name=f"vps{tt}") for tt in tts}
                        for kt in range(KT):
                            for tt in tts:
                                nc.tensor.matmul(
                                    vps[tt][:],
                                    xT_sb[:, kt, tt * 128:(tt + 1) * 128],
                                    wv_sb[:, kt, :],
                                    start=(kt == 0), stop=(kt == KT - 1),
                                )
                        for tt in tts:
                            nc.vector.tensor_scalar_mul(
                                vmask_sb[:, tt, :, :]
                                .rearrange("p h d -> p (h d)"),
                                vps[tt][:], mcol_sb[:, tt:tt + 1])

                    # head schedule: v epoch 0, first two p0 chunks, then the
                    # second v epoch split around the last two chunks so the
                    # PE always has v matmuls while the rope chains drain
                    cl0 = chunk_list(0)
                    v_epoch(0)
                    for (m, ch) in cl0[:2]:
                        ps = emit_qk_chunk_mms(m, ch)
                        st0 = emit_rope_a(m, ch, ps)
                        emit_rope_b(m, ch, *st0)
                    tts1 = range(4, 8)
                    vps1 = {tt: psV.tile([128, 512], F32, tag=f"v{tt % 4}",
                                         name=f"vps{tt}") for tt in tts1}

                    def v1_kts(k0, k1):
                        for kt in range(k0, k1):
                            for tt in tts1:
                                nc.tensor.matmul(
                                    vps1[tt][:],
                                    xT_sb[:, kt, tt * 128:(tt + 1) * 128],
                                    wv_sb[:, kt, :],
                                    start=(kt == 0), stop=(kt == KT - 1),
                                )

                    v1_kts(0, 4)
                    m, ch = cl0[2]
                    ps = emit_qk_chunk_mms(m, ch)
                    st0 = emit_rope_a(m, ch, ps)
                    v1_kts(4, 5)
                    emit_rope_b(m, ch, *st0)
                    v1_kts(5, 6)
                    m, ch = cl0[3]
                    ps = emit_qk_chunk_mms(m, ch)
                    head_tail = (m, ch, emit_rope_a(m, ch, ps))
                    v1_kts(6, 8)
                    for tt in tts1:
                        nc.vector.tensor_scalar_mul(
                            vmask_sb[:, tt, :, :]
                            .rearrange("p h d -> p (h d)"),
                            vps1[tt][:], mcol_sb[:, tt:tt + 1])

                # ---- windows ----
                with (
                    tc.tile_pool(name="psSc", bufs=1, space="PSUM") as psSc,
                    tc.tile_pool(name="psCtx", bufs=1, space="PSUM") as psCtx,
                    tc.tile_pool(name="psDen", bufs=1, space="PSUM") as psDen,
                ):
                    # scores + exp for one (p, tt, ch); the exp scale divides
                    # out the QSC^2 carried by the cos/sin tables
                    def emit_scores(p, tt, ch, seq):
                        qp = rope_sb[:, p, :]
                        kp = rope_sb[:, p + 4, :]
                        ps = psSc.tile([128, 2, 512], F32, tag=f"sc{seq % 2}",
                                       name=f"sc{p}_{tt}_{ch}")
                        for hh in range(2):
                            base = hh * 64
                            nc.tensor.matmul(
                                ps[:, hh, :],
                                kp[base:base + 64, tt * 128:(tt + 1) * 128],
                                qp[base:base + 64, ch * 512:(ch + 1) * 512],
                                start=True, stop=True,
                                tile_position=(base, 0),
                            )
                        ex = expool.tile([128, 2, 512], BF16, tag="ex",
                                         name=f"ex{p}_{tt}_{ch}")
                        nc.scalar.activation(ex[:], ps[:], AF.Exp,
                                             scale=0.125 / (QSC * QSC))
                        return ex

                    # transposed ctx + denominator for one (p, tt)
                    def emit_ctx_den(p, tt, exs, ctx_ps, den_ps):
                        for ch in range(2):
                            ex = exs[ch]
                            for hh in range(2):
                                for jc in range(4):
                                    jj = ch * 4 + jc
                                    lhsT = ex[:, hh, jc * 128:(jc + 1) * 128]
                                    first = (tt == 0 and ch == 0 and jc == 0)
                                    last = (tt == ST - 1 and ch == 1
                                            and jc == 3)
                                    nc.tensor.matmul(
                                        ctx_ps[hh][:, jj, :],
                                        lhsT,
                                        vmask_sb[:, tt, 2 * p + hh, :],
                                        start=first, stop=last,
                                    )
                                    dfirst = (tt == 0 and ch == 0
                                              and hh == 0 and jc == 0)
                                    dlast = (tt == ST - 1 and ch == 1
                                             and hh == 1 and jc == 3)
                                    nc.tensor.matmul(
                                        den_ps[:, hh * 8 + jj:
                                               hh * 8 + jj + 1],
                                        lhsT,
                                        mcolB_sb[:, tt:tt + 1],
                                        start=dfirst, stop=dlast,
                                    )

                    def emit_ctx_norm(p, ctx_ps, den_ps):
                        recip = small.tile([128, 16], F32, tag="recip")
                        nc.vector.reciprocal(recip[:], den_ps[:, 0:16])
                        for hh in range(2):
                            # one broadcast multiply per head: the per-(q,jj)
                            # reciprocal is stride-0 along d (only one PSUM
                            # operand is allowed per DVE instruction)
                            nc.vector.tensor_mul(
                                ctxq_sb[:, p, :, hh * 64:(hh + 1) * 64],
                                ctx_ps[hh][:],
                                recip[:, hh * 8:hh * 8 + 8]
                                .rearrange("p (j o) -> p j o", o=1)
                                .broadcast_to([128, 8, 64]))

                    # transpose one (p, half): 4 PE transposes into a 2KB
                    # psum tile + one copy into ctxT_sb
                    def emit_transpose(pool, tag, pd, half, ceng=None):
                        pst = pool.tile([128, 4, 256], BF16, tag=tag,
                                        name=f"pst{pd}_{half}")
                        for q in range(4):
                            nc.tensor.matmul(
                                pst[:, q, 0:128],
                                ctxq_sb[:, pd, half * 4 + q, :],
                                ident_sb[:],
                                is_transpose=True, start=True, stop=True)
                        dst = ctxT_sb[:, pd, half * 512:(half + 1) * 512] \
                            .rearrange("p (q f) -> p q f", f=128)
                        if ceng is nc.scalar:
                            nc.scalar.copy(dst, pst[:, :, 0:128])
                        else:
                            nc.vector.tensor_copy(dst, pst[:, :, 0:128])

                    # Global software pipeline: the ctx/den stream trails the
                    # scores stream by LAG slots ACROSS window boundaries, so
                    # the last window's drain fills the next window's empty
                    # ctx slots and exp never waits on the PE.
                    seq = 0
                    exs_by = {}
                    prev = None      # (p-1, ctx_ps, den_ps) awaiting drain
                    cur = None
                    # the last head rope chain's perm matmul drops into the
                    # first window slot (its k-tiles aren't needed until tt=4)
                    pending_rope_b = head_tail
                    for p in range(4):
                        nxt = chunk_list(p + 1) if p < 3 else []
                        st = None
                        for tt in range(ST):
                            if tt == LAG:
                                # the ctx tiles' reuse waits on the previous
                                # window's normalization; lead with scores
                                for ch in range(2):
                                    exs_by.setdefault((p, tt), []).append(
                                        emit_scores(p, tt, ch, seq))
                                    seq += 1
                            if tt >= LAG:
                                if cur is None:
                                    cur = (
                                        [psCtx.tile([128, ST, 64], F32,
                                                    tag=f"ctx{hh}",
                                                    name=f"ctx{p}_{hh}")
                                         for hh in range(2)],
                                        psDen.tile([128, 512], F32,
                                                   tag="den", name=f"den{p}"))
                                emit_ctx_den(p, tt - LAG,
                                             exs_by.pop((p, tt - LAG)),
                                             cur[0], cur[1])
                            elif prev is not None:
                                dp, dctx, dden = prev
                                emit_ctx_den(dp, tt + ST - LAG,
                                             exs_by.pop((dp, tt + ST - LAG)),
                                             dctx, dden)
                                if tt == LAG - 1:
                                    emit_ctx_norm(dp, dctx, dden)
                                    prev = None
                            if (p, tt) not in exs_by:
                                exs = []
                                for ch in range(2):
                                    exs.append(emit_scores(p, tt, ch, seq))
                                    seq += 1
                                exs_by[(p, tt)] = exs
                            if pending_rope_b is not None:
                                hm, hch, hst = pending_rope_b
                                emit_rope_b(hm, hch, *hst)
                                pending_rope_b = None
                            # interleave next head-pair's qk + rope chain:
                            # even slot: qk matmuls + bias/cos terms,
                            # odd slot: perm matmul + sin term + sum.
                            # In the last window (no next pair) the freed qk
                            # bank hosts the p<3 context transposes instead.
                            if nxt:
                                m, ch = nxt[tt // 2]
                                if tt % 2 == 0:
                                    ps = emit_qk_chunk_mms(m, ch)
                                    st = emit_rope_a(m, ch, ps)
                                else:
                                    emit_rope_b(m, ch, *st)
                            elif tt < 6:
                                emit_transpose(psQk, "qk", tt // 2, tt % 2)
                            if p == 2 and tt == 0:
                                for kt in range(4):
                                    nc.sync.dma_start(
                                        wout_sb[:, kt, :],
                                        wout[kt * 128:(kt + 1) * 128, :])
                        prev, cur = (p, cur[0], cur[1]), None
                    # final drain for p=3
                    dp, dctx, dden = prev
                    for tt in range(ST - LAG, ST):
                        emit_ctx_den(dp, tt, exs_by.pop((dp, tt)),
                                     dctx, dden)
                    emit_ctx_norm(dp, dctx, dden)

                    # ---- tail (same psum pools: psy reuses the sc tags,
                    # whose last readers — the exps — retire early; the p3
                    # transposes reuse the qk tag) ----
                    with tc.tile_pool(name="ytp", bufs=3) as ytp:
                        def outproj_mms(m, psy, k0, k1, start, stop):
                            for kt in range(k0, k1):
                                for ch in range(2):
                                    nc.tensor.matmul(
                                        psy[:, ch * 512:(ch + 1) * 512],
                                        wout_sb[:, kt,
                                                m * 128:(m + 1) * 128],
                                        ctxT_sb[:, kt,
                                                ch * 512:(ch + 1) * 512],
                                        start=(start and kt == k0),
                                        stop=(stop and kt == k1 - 1))

                        # m0/m1 accumulate their first three kt while the p3
                        # normalization + transposes drain; kt=3 joins after
                        psys = {}
                        for m in range(2):
                            psy = psSc.tile([128, 1024], F32,
                                            tag=f"sc{m % 2}", name=f"psy{m}")
                            psys[m] = psy
                            outproj_mms(m, psy, 0, 3, True, False)
                        emit_transpose(psQk, "qk", 3, 0)
                        emit_transpose(psDen, "den", 3, 1, ceng=nc.scalar)
                        for m in range(8):
                            if m < 2:
                                psy = psys[m]
                                outproj_mms(m, psy, 3, 4, False, True)
                            else:
                                psy = psSc.tile([128, 1024], F32,
                                                tag=f"sc{m % 2}",
                                                name=f"psy{m}")
                                outproj_mms(m, psy, 0, 4, True, True)
                            if m == 7:
                                # split the last tile across engines
                                # (separate tiles so the copies overlap)
                                ya = ytp.tile([128, 512], BF16, tag="ya",
                                              name="yt7a")
                                yb = ytp.tile([128, 512], BF16, tag="yb",
                                              name="yt7b")
                                nc.scalar.copy(ya[:], psy[:, 0:512])
                                nc.vector.tensor_copy(yb[:],
                                                      psy[:, 512:1024])
                                nc.sync.dma_start(
                                    yT[m * 128:(m + 1) * 128, 0:512], ya[:])
                                nc.gpsimd.dma_start(
                                    yT[m * 128:(m + 1) * 128, 512:1024],
                                    yb[:])
                                continue
                            yt = ytp.tile([128, 1024], BF16, tag="yt",
                                          name=f"yt{m}")
                            if m % 2 == 0:
                                nc.scalar.copy(yt[:], psy[:])
                            else:
                                nc.vector.tensor_copy(yt[:], psy[:])
                            deng = nc.sync if m % 2 == 0 else nc.gpsimd
                            deng.dma_start(yT[m * 128:(m + 1) * 128, :],
                                           yt[:])

    return nc


def _split_waits(nc, max_waits=1):
    """This walrus build rejects >1 sync-wait command per instruction; hoist
    extra waits onto preceding NoOps on the same engine/queue."""
    for bb in nc.main_func.blocks:
        new_insts = []
        for ins in bb.instructions:
            si = getattr(ins, "sync_info", None)
            if si is not None and si.on_wait and len(si.on_wait) > max_waits:
                waits = list(si.on_wait)
                head, rest = waits[:max_waits], waits[max_waits:]
                while rest:
                    chunk, rest = rest[:max_waits], rest[max_waits:]
                    new_insts.append(mybir.InstNoOp(
                        name=f"waitsplit-{nc.next_id()}", ins=[], outs=[],
                        sync_info=mybir.SyncInfo(on_wait=chunk, on_update=[]),
                        engine=ins.engine))
                ins.sync_info = mybir.SyncInfo(
                    on_wait=head, on_update=list(si.on_update or []))
            new_insts.append(ins)
        bb.instructions = new_insts


def make_core_inputs(x, attention_mask, Wqkv, bqkv, Wout):
    """Host-side shard prep: returns list of 8 in_maps (core = 2*b + g)."""
    BF = ml_dtypes.bfloat16
    Wr = np.ascontiguousarray(Wqkv).reshape(HID, 3, H, D)
    br = np.ascontiguousarray(bqkv).reshape(3, H, D)

    inv = 1.0 / (THETA ** (np.arange(0, D, 2, dtype=np.float64) / D))
    pos = np.arange(S, dtype=np.float64)
    freqs = pos[:, None] * inv[None, :]              # [S, 32]
    emb = np.concatenate([freqs, freqs], axis=1)     # [S, 64]
    cosT = np.cos(emb).T.astype(np.float32)          # [64, S]
    sgn = np.concatenate([-np.ones(32), np.ones(32)])[:, None]
    sinTs = (sgn * np.sin(emb).T).astype(np.float32)
    # QSC-scaled tables push the fp8 rope values out of e4m3 subnormals;
    # the exp scale divides the QSC^2 back out.
    cos2 = (np.concatenate([cosT, cosT], 0) * 8.0).astype(BF)   # [128, S]
    sin2 = (np.concatenate([sinTs, sinTs], 0) * 8.0).astype(BF)
    ident = np.eye(128, dtype=np.float32).astype(BF)
    pp = np.arange(128)
    shmap = (pp - pp % 64) + (pp % 64 + 32) % 64
    permTm = np.zeros((128, 128), dtype=np.float32)
    permTm[shmap, pp] = 1.0
    permTm = permTm.astype(BF)

    in_maps = []
    for c in range(NCORES):
        b, g = c // 2, c % 2
        hs = slice(g * HPC, (g + 1) * HPC)
        wqk = np.concatenate(
            [Wr[:, 0, hs, :].reshape(HID, 512),
             Wr[:, 1, hs, :].reshape(HID, 512)], axis=1)
        wv = Wr[:, 2, hs, :].reshape(HID, 512)
        bqk = np.concatenate(
            [br[0, hs].reshape(512), br[1, hs].reshape(512)]
        ).reshape(8, 128).T
        mcolv = attention_mask[b].astype(np.float32).reshape(ST, 128).T
        in_maps.append({
            "xT": np.ascontiguousarray(x[b].T.astype(BF)),
            "wqk": np.ascontiguousarray(wqk.astype(BF)),
            "wv": np.ascontiguousarray(wv.astype(BF)),
            "wout": np.ascontiguousarray(
                Wout[g * 512:(g + 1) * 512, :].astype(BF)),
            "bqk": np.ascontiguousarray(bqk.astype(np.float32)),
            "cosT": cos2, "sinT": sin2, "permT": permTm,
            "mcol": np.ascontiguousarray(mcolv),
            "mcolB": np.ascontiguousarray(mcolv.astype(BF)),
            "ident": ident,
        })
    return in_maps


_PROGRAM = None


def kernel(x, attention_mask, Wqkv, bqkv, Wout, bout, _trace=False):
    global _PROGRAM
    x = np.asarray(x)
    attention_mask = np.asarray(attention_mask)
    Wqkv = np.asarray(Wqkv)
    bqkv = np.asarray(bqkv)
    Wout = np.asarray(Wout)
    bout = np.asarray(bout)

    if _PROGRAM is None:
        _PROGRAM = build_program()
        _split_waits(_PROGRAM)
    nc = _PROGRAM

    in_maps = make_core_inputs(x, attention_mask, Wqkv, bqkv, Wout)
    res = run_bass_kernel_spmd(
        nc, in_maps, core_ids=list(range(NCORES)), trace=_trace)

    y = np.empty((B, S, HID), dtype=np.float32)
    for b in range(B):
        acc = (res.results[2 * b]["yT"].astype(np.float32)
               + res.results[2 * b + 1]["yT"].astype(np.float32))
        y[b] = acc.T
    # exact host-side bias corrections: v-bias shifts context by a constant
    # (attn rows sum to 1), q/k biases were applied on device.
    bv = bqkv[2 * HID:3 * HID].astype(np.float32)
    y += (bv @ Wout + bout).astype(np.float32)[None, None, :]
    if _trace:
        kernel.last_exec_time_ns = res.exec_time_ns
    return y


# revision 73
# speedup vs baseline: 1.0582x; 1.0095x over previous
"""DeBERTa-RoPE self-attention on 8 Trainium2 cores.

Sharding: data-parallel over batch (4) x tensor-parallel over heads (2 groups
of 8). Each core computes the qkv projection for its (batch, head-group),
RoPE, attention, and a row-parallel partial out-projection; the host sums the
two partials per batch and applies the (constant) v/out bias correction.

Device schedule (per core), engine-balanced and software-pipelined:
 - head: input DMAs spread over SP/Pool/DVE/ACT queues; v-projection
   (kt-outer, 8 psum banks); qk projection + RoPE for head-pair 0.
 - windows p=0..3: scores (bf16 matmuls) -> exp (ACT, the window pacer)
   -> transposed context accumulation (out[q,d] += ex^T v, 64-row matmuls)
   plus 1-row denominator matmuls, while the qk projection + RoPE chain for
   head-pair p+1 interleaves into the same PE stream.
 - tail: PE transposes of the normalized context back to [d, q], partial
   out-projection, yT DMA.

RoPE's rotate-half is a partition-block shuffle done with SBUF->SBUF DMAs
(sign folded into sinT); softmax skips max-subtraction (|scores| <= ~5);
masking is folded into v and the denominator column.
"""

import numpy as np
import ml_dtypes

import concourse.bass as bass
import concourse.mybir as mybir
import concourse.tile as tile
from concourse.bass_utils import run_bass_kernel_spmd

H = 16
D = 64
HID = H * D
B = 4
S = 1024
THETA = 10000.0
NCORES = 8
HPC = H // 2          # heads per core
KT = HID // 128       # 8 k-tiles
ST = S // 128         # 8 seq tiles
LAG = 5               # ctx trails scores by LAG tt-slots

F32 = mybir.dt.float32
BF16 = mybir.dt.bfloat16
F8 = mybir.dt.float8e4
AF = mybir.ActivationFunctionType
ALU = mybir.AluOpType
DR = mybir.MatmulPerfMode.DoubleRow
QSC = 8.0            # wqk pre-scale (keeps fp8 weights out of subnormals)


def build_program():
    nc = bass.Bass()
    # big inputs come pre-tiled as [128 partitions, kt, cols] so each loads
    # in a couple of large DMAs (the per-DMA queue cost is ~650ns)
    xT = nc.declare_dram_parameter("xT", [128, KT, S], BF16, isOutput=False)
    wqk = nc.declare_dram_parameter("wqk", [128, KT, 1024], BF16,
                                    isOutput=False)
    wv = nc.declare_dram_parameter("wv", [128, KT, 512], BF16, isOutput=False)
    wout = nc.declare_dram_parameter("wout", [128, 4, HID], BF16,
                                     isOutput=False)
    bqk = nc.declare_dram_parameter("bqk", [128, 8], F32, isOutput=False)
    cosT = nc.declare_dram_parameter("cosT", [128, S], BF16, isOutput=False)
    sinT = nc.declare_dram_parameter("sinT", [128, S], BF16, isOutput=False)
    permT = nc.declare_dram_parameter("permT", [128, 128], BF16,
                                      isOutput=False)
    mcol = nc.declare_dram_parameter("mcol", [128, ST], F32, isOutput=False)
    mcolB = nc.declare_dram_parameter("mcolB", [128, ST], BF16, isOutput=False)
    ident = nc.declare_dram_parameter("ident", [128, 128], BF16, isOutput=False)
    yT = nc.declare_dram_parameter("yT", [HID, S], BF16, isOutput=True)

    with tile.TileContext(nc) as tc:
        with (
            tc.tile_pool(name="const", bufs=1) as cpool,
            tc.tile_pool(name="persist", bufs=1) as persist,
        ):
            cos_sb = cpool.tile([128, S], BF16)
            sin_sb = cpool.tile([128, S], BF16)
            mcol_sb = cpool.tile([128, ST], F32)
            mcolB_sb = cpool.tile([128, ST], BF16)
            bqk_sb = cpool.tile([128, 8], F32)
            permT_sb = cpool.tile([128, 128], BF16)
            ident_sb = cpool.tile([128, 128], BF16)

            xT_sb = persist.tile([128, KT, S], BF16)
            wqk_sb = persist.tile([128, KT, 1024], BF16)
            wv_sb = persist.tile([128, KT, 512], BF16)
            wout_sb = persist.tile([128, 4, HID], BF16)
            rope_sb = persist.tile([128, 8, S], BF16)
            vmask_sb = persist.tile([128, ST, HPC, 64], BF16)
            ctxq_sb = persist.tile([128, 4, ST, 128], BF16)
            ctxT_sb = persist.tile([128, 4, S], BF16)

            # ---- input DMAs: few large transfers, chunked so the first
            # matmuls can start while the rest stream in ----
            for kt in range(KT):
                nc.sync.dma_start(xT_sb[:, kt, :], xT[:, kt, :])
                nc.gpsimd.dma_start(wqk_sb[:, kt, :], wqk[:, kt, :])
                nc.scalar.dma_start(wv_sb[:, kt, :], wv[:, kt, :])
            nc.scalar.dma_start(cos_sb[:], cosT[:])
            nc.scalar.dma_start(sin_sb[:], sinT[:])
            nc.scalar.dma_start(mcol_sb[:], mcol[:])
            nc.scalar.dma_start(mcolB_sb[:], mcolB[:])
            nc.scalar.dma_start(bqk_sb[:], bqk[:])
            nc.scalar.dma_start(permT_sb[:], permT[:])
            nc.scalar.dma_start(ident_sb[:], ident[:])

            # ---- attention-era pools (psum: qk 1 + sc 4 + ctx 2 + den 1) ----
            with (
                tc.tile_pool(name="psQk", bufs=1, space="PSUM") as psQk,
                tc.tile_pool(name="rt", bufs=2) as rt,
                tc.tile_pool(name="expool", bufs=12) as expool,
                tc.tile_pool(name="small", bufs=2) as small,
            ):
                # qk projection chunk: 8 accumulating matmuls into one bank
                def emit_qk_chunk_mms(m, ch):
                    ps = psQk.tile([128, 512], F32, tag="qk",
                                   name=f"qk{m}_{ch}")
                    for kt in range(KT):
                        nc.tensor.matmul(
                            ps[:],
                            wqk_sb[:, kt, m * 128:(m + 1) * 128],
                            xT_sb[:, kt, ch * 512:(ch + 1) * 512],
                            start=(kt == 0), stop=(kt == KT - 1),
                        )
                    return ps

                # RoPE part 1 (right after the qk matmuls): bias-add copy to
                # SBUF and the cos-term; both feed part 2.
                def emit_rope_a(m, ch, ps):
                    sl = slice(ch * 512, (ch + 1) * 512)
                    qkb = rt.tile([128, 512], BF16, tag="qkb",
                                  name=f"qkb{m}_{ch}")
                    nc.vector.tensor_scalar_add(
                        qkb[:], ps[:], bqk_sb[:, m:m + 1])
                    t1 = rt.tile([128, 512], BF16, tag="t1")
                    nc.vector.tensor_mul(t1[:], qkb[:], cos_sb[:, sl])
                    return qkb, t1

                # RoPE part 2: rotate-half via perm matmul (reuses the qk
                # bank; bqksh folds in since bqksh = P @ bqk), sin-term, sum
                # (fp8 out), then the pair-shuffle DMA into DoubleRow layout:
                # dims (2a, 2a+1) land in (partition a, slots 0/1).
                def emit_rope_b(m, ch, qkb, t1):
                    sl = slice(ch * 512, (ch + 1) * 512)
                    ps_sh = psQk.tile([128, 512], F32, tag="qk",
                                      name=f"sh{m}_{ch}")
                    nc.tensor.matmul(ps_sh[:], permT_sb[:], qkb[:],
                                     start=True, stop=True)
                    s2 = rt.tile([128, 512], BF16, tag="s2")
                    nc.vector.tensor_mul(s2[:], ps_sh[:], sin_sb[:, sl])
                    nc.vector.tensor_add(rope_sb[:, m, sl], t1[:], s2[:])

                # chunk order per p: k-ch0, q-ch0, q-ch1, k-ch1 (matches the
                # order the consumer scores matmuls need them)
                def chunk_list(p):
                    return [(p + 4, 0), (p, 0), (p, 1), (p + 4, 1)]

                # ---- head: v projection (2 epochs of 4 banks) interleaved
                # with the p=0 qk projection + RoPE ----
                with tc.tile_pool(name="psV", bufs=1, space="PSUM") as psV:
                    def v_epoch(ep):
                        tts = range(ep * 4, ep * 4 + 4)
                        vps = {tt: psV.tile([128, 512], F32, tag=f"v{tt % 4}",
                                            name=f"vps{tt}") for tt in tts}
                        for kt in range(KT):
                            for tt in tts:
                                nc.tensor.matmul(
                                    vps[tt][:],
                                    xT_sb[:, kt, tt * 128:(tt + 1) * 128],
                                    wv_sb[:, kt, :],
                                    start=(kt == 0), stop=(kt == KT - 1),
                                )
                        for tt in tts:
                            nc.vector.tensor_scalar_mul(
                                vmask_sb[:, tt, :, :]
                                .rearrange("p h d -> p (h d)"),
                                vps[tt][:], mcol_sb[:, tt:tt + 1])

                    # head schedule: v epoch 0, first two p0 chunks, then the
                    # second v epoch split around the last two chunks so the
                    # PE always has v matmuls while the rope chains drain
                    cl0 = chunk_list(0)
                    v_epoch(0)
                    for (m, ch) in cl0[:2]:
                        ps = emit_qk_chunk_mms(m, ch)
                        st0 = emit_rope_a(m, ch, ps)
                        emit_rope_b(m, ch, *st0)
                    tts1 = range(4, 8)
                    vps1 = {tt: psV.tile([128, 512], F32, tag=f"v{tt % 4}",
                                         name=f"vps{tt}") for tt in tts1}

                    def v1_kts(k0, k1):
                        for kt in range(k0, k1):
                            for tt in tts1:
                                nc.tensor.matmul(
                                    vps1[tt][:],
                                    xT_sb[:, kt, tt * 128:(tt + 1) * 128],
                                    wv_sb[:, kt, :],
                                    start=(kt == 0), stop=(kt == KT - 1),
                                )

                    v1_kts(0, 4)
                    m, ch = cl0[2]
                    ps = emit_qk_chunk_mms(m, ch)
                    st0 = emit_rope_a(m, ch, ps)
                    v1_kts(4, 5)
                    emit_rope_b(m, ch, *st0)
                    v1_kts(5, 6)
                    m, ch = cl0[3]
                    ps = emit_qk_chunk_mms(m, ch)
                    head_tail = (m, ch, emit_rope_a(m, ch, ps))
                    v1_kts(6, 8)
                    for tt in tts1:
                        nc.vector.tensor_scalar_mul(
                            vmask_sb[:, tt, :, :]
                            .rearrange("p h d -> p (h d)"),
                            vps1[tt][:], mcol_sb[:, tt:tt + 1])

                # ---- windows ----
                with (
                    tc.tile_pool(name="psSc", bufs=1, space="PSUM") as psSc,
                    tc.tile_pool(name="psCtx", bufs=1, space="PSUM") as psCtx,
                    tc.tile_pool(name="psDen", bufs=1, space="PSUM") as psDen,
                ):
                    # scores + exp for one (p, tt, ch); the exp scale divides
                    # out the QSC^2 carried by the cos/sin tables
                    def emit_scores(p, tt, ch, seq):
                        qp = rope_sb[:, p, :]
                        kp = rope_sb[:, p + 4, :]
                        ps = psSc.tile([128, 2, 512], F32, tag=f"sc{seq % 2}",
                                       name=f"sc{p}_{tt}_{ch}")
                        for hh in range(2):
                            base = hh * 64
                            nc.tensor.matmul(
                                ps[:, hh, :],
                                kp[base:base + 64, tt * 128:(tt + 1) * 128],
                                qp[base:base + 64, ch * 512:(ch + 1) * 512],
                                start=True, stop=True,
                                tile_position=(base, 0),
                            )
                        ex = expool.tile([128, 2, 512], BF16, tag="ex",
                                         name=f"ex{p}_{tt}_{ch}")
                        nc.scalar.activation(ex[:], ps[:], AF.Exp,
                                             scale=0.125 / (QSC * QSC))
                        return ex

                    # transposed ctx + denominator for one (p, tt)
                    def emit_ctx_den(p, tt, exs, ctx_ps, den_ps):
                        for ch in range(2):
                            ex = exs[ch]
                            for hh in range(2):
                                for jc in range(4):
                                    jj = ch * 4 + jc
                                    lhsT = ex[:, hh, jc * 128:(jc + 1) * 128]
                                    first = (tt == 0 and ch == 0 and jc == 0)
                                    last = (tt == ST - 1 and ch == 1
                                            and jc == 3)
                                    nc.tensor.matmul(
                                        ctx_ps[hh][:, jj, :],
                                        lhsT,
                                        vmask_sb[:, tt, 2 * p + hh, :],
                                        start=first, stop=last,
                                    )
                                    dfirst = (tt == 0 and ch == 0
                                              and hh == 0 and jc == 0)
                                    dlast = (tt == ST - 1 and ch == 1
                                             and hh == 1 and jc == 3)
                                    nc.tensor.matmul(
                                        den_ps[:, hh * 8 + jj:
                                               hh * 8 + jj + 1],
                                        lhsT,
                                        mcolB_sb[:, tt:tt + 1],
                                        start=dfirst, stop=dlast,
                                    )

                    def emit_ctx_norm(p, ctx_ps, den_ps):
                        recip = small.tile([128, 16], F32, tag="recip")
                        nc.vector.reciprocal(recip[:], den_ps[:, 0:16])
                        for hh in range(2):
                            # one broadcast multiply per head: the per-(q,jj)
                            # reciprocal is stride-0 along d (only one PSUM
                            # operand is allowed per DVE instruction)
                            nc.vector.tensor_mul(
                                ctxq_sb[:, p, :, hh * 64:(hh + 1) * 64],
                                ctx_ps[hh][:],
                                recip[:, hh * 8:hh * 8 + 8]
                                .rearrange("p (j o) -> p j o", o=1)
                                .broadcast_to([128, 8, 64]))

                    # transpose one (p, half): 4 PE transposes into a 2KB
                    # psum tile + one copy into ctxT_sb
                    def emit_transpose(pool, tag, pd, half, ceng=None):
                        pst = pool.tile([128, 4, 256], BF16, tag=tag,
                                        name=f"pst{pd}_{half}")
                        for q in range(4):
                            nc.tensor.matmul(
                                pst[:, q, 0:128],
                                ctxq_sb[:, pd, half * 4 + q, :],
                                ident_sb[:],
                                is_transpose=True, start=True, stop=True)
                        dst = ctxT_sb[:, pd, half * 512:(half + 1) * 512] \
                            .rearrange("p (q f) -> p q f", f=128)
                        if ceng is nc.scalar:
                            nc.scalar.copy(dst, pst[:, :, 0:128])
                        else:
                            nc.vector.tensor_copy(dst, pst[:, :, 0:128])

                    # Global software pipeline: the ctx/den stream trails the
                    # scores stream by LAG slots ACROSS window boundaries, so
                    # the last window's drain fills the next window's empty
                    # ctx slots and exp never waits on the PE.
                    seq = 0
                    exs_by = {}
                    prev = None      # (p-1, ctx_ps, den_ps) awaiting drain
                    cur = None
                    # the last head rope chain's perm matmul drops into the
                    # first window slot (its k-tiles aren't needed until tt=4)
                    pending_rope_b = head_tail
                    for p in range(4):
                        nxt = chunk_list(p + 1) if p < 3 else []
                        st = None
                        for tt in range(ST):
                            if tt == LAG:
                                # the ctx tiles' reuse waits on the previous
                                # window's normalization; lead with scores
                                for ch in range(2):
                                    exs_by.setdefault((p, tt), []).append(
                                        emit_scores(p, tt, ch, seq))
                                    seq += 1
                            if tt >= LAG:
                                if cur is None:
                                    cur = (
                                        [psCtx.tile([128, ST, 64], F32,
                                                    tag=f"ctx{hh}",
                                                    name=f"ctx{p}_{hh}")
                                         for hh in range(2)],
                                        psDen.tile([128, 512], F32,
                                                   tag="den", name=f"den{p}"))
                                emit_ctx_den(p, tt - LAG,
                                             exs_by.pop((p, tt - LAG)),
                                             cur[0], cur[1])
                            elif prev is not None:
                                dp, dctx, dden = prev
                                emit_ctx_den(dp, tt + ST - LAG,
                                             exs_by.pop((dp, tt + ST - LAG)),
                                             dctx, dden)
                                if tt == LAG - 1:
                                    emit_ctx_norm(dp, dctx, dden)
                                    prev = None
                            if (p, tt) not in exs_by:
                                exs = []
                                for ch in range(2):
                                    exs.append(emit_scores(p, tt, ch, seq))
                                    seq += 1
                                exs_by[(p, tt)] = exs
                            if pending_rope_b is not None:
                                hm, hch, hst = pending_rope_b
                                emit_rope_b(hm, hch, *hst)
                                pending_rope_b = None
                            # interleave next head-pair's qk + rope chain:
                            # even slot: qk matmuls + bias/cos terms,
                            # odd slot: perm matmul + sin term + sum.
                            # In the last window (no next pair) the freed qk
                            # bank hosts the p<3 context transposes instead.
                            if nxt:
                                m, ch = nxt[tt // 2]
                                if tt % 2 == 0:
                                    ps = emit_qk_chunk_mms(m, ch)
                                    st = emit_rope_a(m, ch, ps)
                                else:
                                    emit_rope_b(m, ch, *st)
                            elif tt >= 2:
                                emit_transpose(psQk, "qk", (tt - 2) // 2,
                                               tt % 2)
                            if p == 2 and tt == 0:
                                nc.sync.dma_start(wout_sb[:], wout[:])
                        prev, cur = (p, cur[0], cur[1]), None
                    # final drain for p=3
                    dp, dctx, dden = prev
                    for tt in range(ST - LAG, ST):
                        emit_ctx_den(dp, tt, exs_by.pop((dp, tt)),
                                     dctx, dden)
                    emit_ctx_norm(dp, dctx, dden)

                    # ---- tail (same psum pools: psy reuses the sc tags,
                    # whose last readers — the exps — retire early; the p3
                    # transposes reuse the qk tag) ----
                    with tc.tile_pool(name="ytp", bufs=3) as ytp:
                        def outproj_mms(m, psy, k0, k1, start, stop):
                            for kt in range(k0, k1):
                                for ch in range(2):
                                    nc.tensor.matmul(
                                        psy[:, ch * 512:(ch + 1) * 512],
                                        wout_sb[:, kt,
                                                m * 128:(m + 1) * 128],
                                        ctxT_sb[:, kt,
                                                ch * 512:(ch + 1) * 512],
                                        start=(start and kt == k0),
                                        stop=(stop and kt == k1 - 1))

                        # m0/m1 accumulate their first three kt while the p3
                        # normalization + transposes drain; kt=3 joins after
                        psys = {}
                        for m in range(2):
                            psy = psSc.tile([128, 1024], F32,
                                            tag=f"sc{m % 2}", name=f"psy{m}")
                            psys[m] = psy
                            outproj_mms(m, psy, 0, 3, True, False)
                        emit_transpose(psQk, "qk", 3, 0)
                        emit_transpose(psDen, "den", 3, 1, ceng=nc.scalar)
                        for m in range(8):
                            if m < 2:
                                psy = psys[m]
                                outproj_mms(m, psy, 3, 4, False, True)
                            else:
                                psy = psSc.tile([128, 1024], F32,
                                                tag=f"sc{m % 2}",
                                                name=f"psy{m}")
                                outproj_mms(m, psy, 0, 4, True, True)
                            if m == 7:
                                # split the last tile across engines
                                # (separate tiles so the copies overlap)
                                ya = ytp.tile([128, 512], BF16, tag="ya",
                                              name="yt7a")
                                yb = ytp.tile([128, 512], BF16, tag="yb",
                                              name="yt7b")
                                nc.scalar.copy(ya[:], psy[:, 0:512])
                                nc.vector.tensor_copy(yb[:],
                                                      psy[:, 512:1024])
                                nc.sync.dma_start(
                                    yT[m * 128:(m + 1) * 128, 0:512], ya[:])
                                nc.gpsimd.dma_start(
                                    yT[m * 128:(m + 1) * 128, 512:1024],
                                    yb[:])
                                continue
                            yt = ytp.tile([128, 1024], BF16, tag="yt",
                                          name=f"yt{m}")
                            if m % 2 == 0:
                                nc.scalar.copy(yt[:], psy[:])
                            else:
                                nc.vector.tensor_copy(yt[:], psy[:])
                            deng = nc.sync if m % 2 == 0 else nc.gpsimd
                            deng.dma_start(yT[m * 128:(m + 1) * 128, :],
                                           yt[:])

    return nc


def _split_waits(nc, max_waits=1):
    """This walrus build rejects >1 sync-wait command per instruction; hoist
    extra waits onto preceding NoOps on the same engine/queue."""
    for bb in nc.main_func.blocks:
        new_insts = []
        for ins in bb.instructions:
            si = getattr(ins, "sync_info", None)
            if si is not None and si.on_wait and len(si.on_wait) > max_waits:
                waits = list(si.on_wait)
                head, rest = waits[:max_waits], waits[max_waits:]
                while rest:
                    chunk, rest = rest[:max_waits], rest[max_waits:]
                    new_insts.append(mybir.InstNoOp(
                        name=f"waitsplit-{nc.next_id()}", ins=[], outs=[],
                        sync_info=mybir.SyncInfo(on_wait=chunk, on_update=[]),
                        engine=ins.engine))
                ins.sync_info = mybir.SyncInfo(
                    on_wait=head, on_update=list(si.on_update or []))
            new_insts.append(ins)
        bb.instructions = new_insts


def make_core_inputs(x, attention_mask, Wqkv, bqkv, Wout):
    """Host-side shard prep: returns list of 8 in_maps (core = 2*b + g)."""
    BF = ml_dtypes.bfloat16
    Wr = np.ascontiguousarray(Wqkv).reshape(HID, 3, H, D)
    br = np.ascontiguousarray(bqkv).reshape(3, H, D)

    inv = 1.0 / (THETA ** (np.arange(0, D, 2, dtype=np.float64) / D))
    pos = np.arange(S, dtype=np.float64)
    freqs = pos[:, None] * inv[None, :]              # [S, 32]
    emb = np.concatenate([freqs, freqs], axis=1)     # [S, 64]
    cosT = np.cos(emb).T.astype(np.float32)          # [64, S]
    sgn = np.concatenate([-np.ones(32), np.ones(32)])[:, None]
    sinTs = (sgn * np.sin(emb).T).astype(np.float32)
    # QSC-scaled tables push the fp8 rope values out of e4m3 subnormals;
    # the exp scale divides the QSC^2 back out.
    cos2 = (np.concatenate([cosT, cosT], 0) * 8.0).astype(BF)   # [128, S]
    sin2 = (np.concatenate([sinTs, sinTs], 0) * 8.0).astype(BF)
    ident = np.eye(128, dtype=np.float32).astype(BF)
    pp = np.arange(128)
    shmap = (pp - pp % 64) + (pp % 64 + 32) % 64
    permTm = np.zeros((128, 128), dtype=np.float32)
    permTm[shmap, pp] = 1.0
    permTm = permTm.astype(BF)

    in_maps = []
    for c in range(NCORES):
        b, g = c // 2, c % 2
        hs = slice(g * HPC, (g + 1) * HPC)
        wqk = np.concatenate(
            [Wr[:, 0, hs, :].reshape(HID, 512),
             Wr[:, 1, hs, :].reshape(HID, 512)], axis=1)
        wv = Wr[:, 2, hs, :].reshape(HID, 512)
        bqk = np.concatenate(
            [br[0, hs].reshape(512), br[1, hs].reshape(512)]
        ).reshape(8, 128).T
        mcolv = attention_mask[b].astype(np.float32).reshape(ST, 128).T

        def tile128(a):  # [kt*128, c] -> [128, kt, c]
            return np.ascontiguousarray(
                a.reshape(-1, 128, a.shape[1]).transpose(1, 0, 2).astype(BF))

        in_maps.append({
            "xT": tile128(x[b].T),
            "wqk": tile128(wqk),
            "wv": tile128(wv),
            "wout": tile128(Wout[g * 512:(g + 1) * 512, :]),
            "bqk": np.ascontiguousarray(bqk.astype(np.float32)),
            "cosT": cos2, "sinT": sin2, "permT": permTm,
            "mcol": np.ascontiguousarray(mcolv),
            "mcolB": np.ascontiguousarray(mcolv.astype(BF)),
            "ident": ident,
        })
    return in_maps


_PROGRAM = None


def kernel(x, attention_mask, Wqkv, bqkv, Wout, bout, _trace=False):
    global _PROGRAM
    x = np.asarray(x)
    attention_mask = np.asarray(attention_mask)
    Wqkv = np.asarray(Wqkv)
    bqkv = np.asarray(bqkv)
    Wout = np.asarray(Wout)
    bout = np.asarray(bout)

    if _PROGRAM is None:
        _PROGRAM = build_program()
        _split_waits(_PROGRAM)
    nc = _PROGRAM

    in_maps = make_core_inputs(x, attention_mask, Wqkv, bqkv, Wout)
    res = run_bass_kernel_spmd(
        nc, in_maps, core_ids=list(range(NCORES)), trace=_trace)

    y = np.empty((B, S, HID), dtype=np.float32)
    for b in range(B):
        acc = (res.results[2 * b]["yT"].astype(np.float32)
               + res.results[2 * b + 1]["yT"].astype(np.float32))
        y[b] = acc.T
    # exact host-side bias corrections: v-bias shifts context by a constant
    # (attn rows sum to 1), q/k biases were applied on device.
    bv = bqkv[2 * HID:3 * HID].astype(np.float32)
    y += (bv @ Wout + bout).astype(np.float32)[None, None, :]
    if _trace:
        kernel.last_exec_time_ns = res.exec_time_ns
    return y


# revision 75
# speedup vs baseline: 1.0725x; 1.0136x over previous
"""DeBERTa-RoPE self-attention on 8 Trainium2 cores.

Sharding: data-parallel over batch (4) x tensor-parallel over heads (2 groups
of 8). Each core computes the qkv projection for its (batch, head-group),
RoPE, attention, and a row-parallel partial out-projection; the host sums the
two partials per batch and applies the (constant) v/out bias correction.

Device schedule (per core), engine-balanced and software-pipelined:
 - head: input DMAs spread over SP/Pool/DVE/ACT queues; v-projection
   (kt-outer, 8 psum banks); qk projection + RoPE for head-pair 0.
 - windows p=0..3: scores (bf16 matmuls) -> exp (ACT, the window pacer)
   -> transposed context accumulation (out[q,d] += ex^T v, 64-row matmuls)
   plus 1-row denominator matmuls, while the qk projection + RoPE chain for
   head-pair p+1 interleaves into the same PE stream.
 - tail: PE transposes of the normalized context back to [d, q], partial
   out-projection, yT DMA.

RoPE's rotate-half is a partition-block shuffle done with SBUF->SBUF DMAs
(sign folded into sinT); softmax skips max-subtraction (|scores| <= ~5);
masking is folded into v and the denominator column.
"""

import numpy as np
import ml_dtypes

import concourse.bass as bass
import concourse.mybir as mybir
import concourse.tile as tile
from concourse.bass_utils import run_bass_kernel_spmd

H = 16
D = 64
HID = H * D
B = 4
S = 1024
THETA = 10000.0
NCORES = 8
HPC = H // 2          # heads per core
KT = HID // 128       # 8 k-tiles
ST = S // 128         # 8 seq tiles
LAG = 5               # ctx trails scores by LAG tt-slots

F32 = mybir.dt.float32
BF16 = mybir.dt.bfloat16
F8 = mybir.dt.float8e4
AF = mybir.ActivationFunctionType
ALU = mybir.AluOpType
DR = mybir.MatmulPerfMode.DoubleRow
QSC = 8.0            # wqk pre-scale (keeps fp8 weights out of subnormals)


def build_program():
    nc = bass.Bass()
    # big inputs come pre-tiled as [128 partitions, kt, cols] so each loads
    # in a couple of large DMAs (the per-DMA queue cost is ~650ns)
    xT = nc.declare_dram_parameter("xT", [128, KT, S], BF16, isOutput=False)
    wqk = nc.declare_dram_parameter("wqk", [128, KT, 1024], BF16,
                                    isOutput=False)
    wv = nc.declare_dram_parameter("wv", [128, KT, 512], BF16, isOutput=False)
    wout = nc.declare_dram_parameter("wout", [128, 4, HID], BF16,
                                     isOutput=False)
    bqk = nc.declare_dram_parameter("bqk", [128, 8], F32, isOutput=False)
    cosT = nc.declare_dram_parameter("cosT", [128, S], BF16, isOutput=False)
    sinT = nc.declare_dram_parameter("sinT", [128, S], BF16, isOutput=False)
    permT = nc.declare_dram_parameter("permT", [128, 128], BF16,
                                      isOutput=False)
    mcol = nc.declare_dram_parameter("mcol", [128, ST], F32, isOutput=False)
    mcolB = nc.declare_dram_parameter("mcolB", [128, ST], BF16, isOutput=False)
    ident = nc.declare_dram_parameter("ident", [128, 128], BF16, isOutput=False)
    yT = nc.declare_dram_parameter("yT", [HID, S], BF16, isOutput=True)

    with tile.TileContext(nc) as tc:
        with (
            tc.tile_pool(name="const", bufs=1) as cpool,
            tc.tile_pool(name="persist", bufs=1) as persist,
        ):
            cos_sb = cpool.tile([128, S], BF16)
            sin_sb = cpool.tile([128, S], BF16)
            mcol_sb = cpool.tile([128, ST], F32)
            mcolB_sb = cpool.tile([128, ST], BF16)
            bqk_sb = cpool.tile([128, 8], F32)
            permT_sb = cpool.tile([128, 128], BF16)
            ident_sb = cpool.tile([128, 128], BF16)

            xT_sb = persist.tile([128, KT, S], BF16)
            wqk_sb = persist.tile([128, KT, 1024], BF16)
            wv_sb = persist.tile([128, KT, 512], BF16)
            wout_sb = persist.tile([128, 4, HID], BF16)
            rope_sb = persist.tile([128, 8, S], BF16)
            vmask_sb = persist.tile([128, ST, HPC, 64], BF16)
            ctxq_sb = persist.tile([128, 4, ST, 128], BF16)
            ctxT_sb = persist.tile([128, 4, S], BF16)

            # ---- input DMAs: few large transfers, chunked so the first
            # matmuls can start while the rest stream in ----
            for kt in range(KT):
                nc.sync.dma_start(xT_sb[:, kt, :], xT[:, kt, :])
                nc.gpsimd.dma_start(wqk_sb[:, kt, :], wqk[:, kt, :])
                nc.scalar.dma_start(wv_sb[:, kt, :], wv[:, kt, :])
            nc.scalar.dma_start(cos_sb[:], cosT[:])
            nc.scalar.dma_start(sin_sb[:], sinT[:])
            nc.scalar.dma_start(mcol_sb[:], mcol[:])
            nc.scalar.dma_start(mcolB_sb[:], mcolB[:])
            nc.scalar.dma_start(bqk_sb[:], bqk[:])
            nc.scalar.dma_start(permT_sb[:], permT[:])
            nc.scalar.dma_start(ident_sb[:], ident[:])

            # ---- attention-era pools (psum: qk 1 + sc 4 + ctx 2 + den 1) ----
            with (
                tc.tile_pool(name="psQk", bufs=1, space="PSUM") as psQk,
                tc.tile_pool(name="rt", bufs=2) as rt,
                tc.tile_pool(name="expool", bufs=12) as expool,
                tc.tile_pool(name="small", bufs=2) as small,
            ):
                # qk projection chunk: 8 accumulating matmuls into one bank
                def emit_qk_chunk_mms(m, ch):
                    ps = psQk.tile([128, 512], F32, tag="qk",
                                   name=f"qk{m}_{ch}")
                    for kt in range(KT):
                        nc.tensor.matmul(
                            ps[:],
                            wqk_sb[:, kt, m * 128:(m + 1) * 128],
                            xT_sb[:, kt, ch * 512:(ch + 1) * 512],
                            start=(kt == 0), stop=(kt == KT - 1),
                        )
                    return ps

                # RoPE part 1 (right after the qk matmuls): bias-add copy to
                # SBUF and the cos-term; both feed part 2.
                def emit_rope_a(m, ch, ps):
                    sl = slice(ch * 512, (ch + 1) * 512)
                    qkb = rt.tile([128, 512], BF16, tag="qkb",
                                  name=f"qkb{m}_{ch}")
                    nc.vector.tensor_scalar_add(
                        qkb[:], ps[:], bqk_sb[:, m:m + 1])
                    t1 = rt.tile([128, 512], BF16, tag="t1")
                    nc.vector.tensor_mul(t1[:], qkb[:], cos_sb[:, sl])
                    return qkb, t1

                # RoPE part 2: rotate-half via perm matmul (reuses the qk
                # bank; bqksh folds in since bqksh = P @ bqk), sin-term, sum
                # (fp8 out), then the pair-shuffle DMA into DoubleRow layout:
                # dims (2a, 2a+1) land in (partition a, slots 0/1).
                def emit_rope_b(m, ch, qkb, t1):
                    sl = slice(ch * 512, (ch + 1) * 512)
                    ps_sh = psQk.tile([128, 512], F32, tag="qk",
                                      name=f"sh{m}_{ch}")
                    nc.tensor.matmul(ps_sh[:], permT_sb[:], qkb[:],
                                     start=True, stop=True)
                    s2 = rt.tile([128, 512], BF16, tag="s2")
                    nc.vector.tensor_mul(s2[:], ps_sh[:], sin_sb[:, sl])
                    nc.vector.tensor_add(rope_sb[:, m, sl], t1[:], s2[:])

                # chunk order per p: k-ch0, q-ch0, q-ch1, k-ch1 (matches the
                # order the consumer scores matmuls need them)
                def chunk_list(p):
                    return [(p + 4, 0), (p, 0), (p, 1), (p + 4, 1)]

                # ---- head: v projection (2 epochs of 4 banks) interleaved
                # with the p=0 qk projection + RoPE ----
                with tc.tile_pool(name="psV", bufs=1, space="PSUM") as psV:
                    def v_epoch(ep):
                        tts = range(ep * 4, ep * 4 + 4)
                        vps = {tt: psV.tile([128, 512], F32, tag=f"v{tt % 4}",
                                            name=f"vps{tt}") for tt in tts}
                        for kt in range(KT):
                            for tt in tts:
                                nc.tensor.matmul(
                                    vps[tt][:],
                                    xT_sb[:, kt, tt * 128:(tt + 1) * 128],
                                    wv_sb[:, kt, :],
                                    start=(kt == 0), stop=(kt == KT - 1),
                                )
                        for tt in tts:
                            nc.vector.tensor_scalar_mul(
                                vmask_sb[:, tt, :, :]
                                .rearrange("p h d -> p (h d)"),
                                vps[tt][:], mcol_sb[:, tt:tt + 1])

                    # head schedule: v epoch 0, first two p0 chunks, then the
                    # second v epoch split around the last two chunks so the
                    # PE always has v matmuls while the rope chains drain
                    cl0 = chunk_list(0)
                    v_epoch(0)
                    for (m, ch) in cl0[:2]:
                        ps = emit_qk_chunk_mms(m, ch)
                        st0 = emit_rope_a(m, ch, ps)
                        emit_rope_b(m, ch, *st0)
                    tts1 = range(4, 6)
                    vps1 = {tt: psV.tile([128, 512], F32, tag=f"v{tt % 4}",
                                         name=f"vps{tt}") for tt in tts1}

                    def v1_kts(k0, k1):
                        for kt in range(k0, k1):
                            for tt in tts1:
                                nc.tensor.matmul(
                                    vps1[tt][:],
                                    xT_sb[:, kt, tt * 128:(tt + 1) * 128],
                                    wv_sb[:, kt, :],
                                    start=(kt == 0), stop=(kt == KT - 1),
                                )

                    v1_kts(0, 4)
                    m, ch = cl0[2]
                    ps = emit_qk_chunk_mms(m, ch)
                    st0 = emit_rope_a(m, ch, ps)
                    v1_kts(4, 5)
                    emit_rope_b(m, ch, *st0)
                    v1_kts(5, 6)
                    m, ch = cl0[3]
                    ps = emit_qk_chunk_mms(m, ch)
                    head_tail = (m, ch, emit_rope_a(m, ch, ps))
                    v1_kts(6, 8)
                    for tt in tts1:
                        nc.vector.tensor_scalar_mul(
                            vmask_sb[:, tt, :, :]
                            .rearrange("p h d -> p (h d)"),
                            vps1[tt][:], mcol_sb[:, tt:tt + 1])

                # ---- windows ----
                with (
                    tc.tile_pool(name="psSc", bufs=1, space="PSUM") as psSc,
                    tc.tile_pool(name="psCtx", bufs=1, space="PSUM") as psCtx,
                    tc.tile_pool(name="psDen", bufs=1, space="PSUM") as psDen,
                ):
                    # scores + exp for one (p, tt, ch); the exp scale divides
                    # out the QSC^2 carried by the cos/sin tables
                    def emit_scores(p, tt, ch, seq):
                        qp = rope_sb[:, p, :]
                        kp = rope_sb[:, p + 4, :]
                        ps = psSc.tile([128, 2, 512], F32, tag=f"sc{seq % 2}",
                                       name=f"sc{p}_{tt}_{ch}")
                        for hh in range(2):
                            base = hh * 64
                            nc.tensor.matmul(
                                ps[:, hh, :],
                                kp[base:base + 64, tt * 128:(tt + 1) * 128],
                                qp[base:base + 64, ch * 512:(ch + 1) * 512],
                                start=True, stop=True,
                                tile_position=(base, 0),
                            )
                        ex = expool.tile([128, 2, 512], BF16, tag="ex",
                                         name=f"ex{p}_{tt}_{ch}")
                        nc.scalar.activation(ex[:], ps[:], AF.Exp,
                                             scale=0.125 / (QSC * QSC))
                        return ex

                    # transposed ctx + denominator for one (p, tt)
                    def emit_ctx_den(p, tt, exs, ctx_ps, den_ps):
                        for ch in range(2):
                            ex = exs[ch]
                            for hh in range(2):
                                for jc in range(4):
                                    jj = ch * 4 + jc
                                    lhsT = ex[:, hh, jc * 128:(jc + 1) * 128]
                                    first = (tt == 0 and ch == 0 and jc == 0)
                                    last = (tt == ST - 1 and ch == 1
                                            and jc == 3)
                                    nc.tensor.matmul(
                                        ctx_ps[hh][:, jj, :],
                                        lhsT,
                                        vmask_sb[:, tt, 2 * p + hh, :],
                                        start=first, stop=last,
                                    )
                                    dfirst = (tt == 0 and ch == 0
                                              and hh == 0 and jc == 0)
                                    dlast = (tt == ST - 1 and ch == 1
                                             and hh == 1 and jc == 3)
                                    nc.tensor.matmul(
                                        den_ps[:, hh * 8 + jj:
                                               hh * 8 + jj + 1],
                                        lhsT,
                                        mcolB_sb[:, tt:tt + 1],
                                        start=dfirst, stop=dlast,
                                    )

                    def emit_ctx_norm(p, ctx_ps, den_ps):
                        recip = small.tile([128, 16], F32, tag="recip")
                        nc.vector.reciprocal(recip[:], den_ps[:, 0:16])
                        for hh in range(2):
                            # one broadcast multiply per head: the per-(q,jj)
                            # reciprocal is stride-0 along d (only one PSUM
                            # operand is allowed per DVE instruction)
                            nc.vector.tensor_mul(
                                ctxq_sb[:, p, :, hh * 64:(hh + 1) * 64],
                                ctx_ps[hh][:],
                                recip[:, hh * 8:hh * 8 + 8]
                                .rearrange("p (j o) -> p j o", o=1)
                                .broadcast_to([128, 8, 64]))

                    # transpose one (p, half): 4 PE transposes into a 2KB
                    # psum tile + one copy into ctxT_sb
                    def emit_transpose(pool, tag, pd, half, ceng=None):
                        pst = pool.tile([128, 4, 256], BF16, tag=tag,
                                        name=f"pst{pd}_{half}")
                        for q in range(4):
                            nc.tensor.matmul(
                                pst[:, q, 0:128],
                                ctxq_sb[:, pd, half * 4 + q, :],
                                ident_sb[:],
                                is_transpose=True, start=True, stop=True)
                        dst = ctxT_sb[:, pd, half * 512:(half + 1) * 512] \
                            .rearrange("p (q f) -> p q f", f=128)
                        if ceng is nc.scalar:
                            nc.scalar.copy(dst, pst[:, :, 0:128])
                        else:
                            nc.vector.tensor_copy(dst, pst[:, :, 0:128])

                    # Global software pipeline: the ctx/den stream trails the
                    # scores stream by LAG slots ACROSS window boundaries, so
                    # the last window's drain fills the next window's empty
                    # ctx slots and exp never waits on the PE.
                    seq = 0
                    exs_by = {}
                    prev = None      # (p-1, ctx_ps, den_ps) awaiting drain
                    cur = None
                    # the last head rope chain's perm matmul drops into the
                    # first window slot (its k-tiles aren't needed until tt=4)
                    pending_rope_b = head_tail
                    for p in range(4):
                        nxt = chunk_list(p + 1) if p < 3 else []
                        st = None
                        for tt in range(ST):
                            if tt == LAG:
                                # the ctx tiles' reuse waits on the previous
                                # window's normalization; lead with scores
                                for ch in range(2):
                                    exs_by.setdefault((p, tt), []).append(
                                        emit_scores(p, tt, ch, seq))
                                    seq += 1
                            if tt >= LAG:
                                if cur is None:
                                    cur = (
                                        [psCtx.tile([128, ST, 64], F32,
                                                    tag=f"ctx{hh}",
                                                    name=f"ctx{p}_{hh}")
                                         for hh in range(2)],
                                        psDen.tile([128, 512], F32,
                                                   tag="den", name=f"den{p}"))
                                emit_ctx_den(p, tt - LAG,
                                             exs_by.pop((p, tt - LAG)),
                                             cur[0], cur[1])
                            elif prev is not None:
                                dp, dctx, dden = prev
                                emit_ctx_den(dp, tt + ST - LAG,
                                             exs_by.pop((dp, tt + ST - LAG)),
                                             dctx, dden)
                                if tt == LAG - 1:
                                    emit_ctx_norm(dp, dctx, dden)
                                    prev = None
                            elif p == 0 and tt < 4:
                                # the last two v tiles run in window 0's
                                # still-idle ctx banks
                                vt = 6 + tt // 2
                                if tt % 2 == 0:
                                    vwin = psCtx.tile(
                                        [128, 512], F32, tag=f"ctx{vt % 2}",
                                        name=f"vps{vt}")
                                    vkts = (0, 4)
                                else:
                                    vkts = (4, 8)
                                for kt in range(*vkts):
                                    nc.tensor.matmul(
                                        vwin[:],
                                        xT_sb[:, kt,
                                              vt * 128:(vt + 1) * 128],
                                        wv_sb[:, kt, :],
                                        start=(kt == 0), stop=(kt == KT - 1))
                                if tt % 2 == 1:
                                    nc.vector.tensor_scalar_mul(
                                        vmask_sb[:, vt, :, :]
                                        .rearrange("p h d -> p (h d)"),
                                        vwin[:], mcol_sb[:, vt:vt + 1])
                            if (p, tt) not in exs_by:
                                exs = []
                                for ch in range(2):
                                    exs.append(emit_scores(p, tt, ch, seq))
                                    seq += 1
                                exs_by[(p, tt)] = exs
                            if pending_rope_b is not None:
                                hm, hch, hst = pending_rope_b
                                emit_rope_b(hm, hch, *hst)
                                pending_rope_b = None
                            # interleave next head-pair's qk + rope chain:
                            # even slot: qk matmuls + bias/cos terms,
                            # odd slot: perm matmul + sin term + sum.
                            # In the last window (no next pair) the freed qk
                            # bank hosts the p<3 context transposes instead.
                            if nxt:
                                m, ch = nxt[tt // 2]
                                if tt % 2 == 0:
                                    ps = emit_qk_chunk_mms(m, ch)
                                    st = emit_rope_a(m, ch, ps)
                                else:
                                    emit_rope_b(m, ch, *st)
                            elif tt >= 2:
                                emit_transpose(psQk, "qk", (tt - 2) // 2,
                                               tt % 2)
                            if p == 2 and tt == 0:
                                nc.sync.dma_start(wout_sb[:], wout[:])
                        prev, cur = (p, cur[0], cur[1]), None
                    # final drain for p=3
                    dp, dctx, dden = prev
                    for tt in range(ST - LAG, ST):
                        emit_ctx_den(dp, tt, exs_by.pop((dp, tt)),
                                     dctx, dden)
                    emit_ctx_norm(dp, dctx, dden)

                    # ---- tail (same psum pools: psy reuses the sc tags,
                    # whose last readers — the exps — retire early; the p3
                    # transposes reuse the qk tag) ----
                    with tc.tile_pool(name="ytp", bufs=3) as ytp:
                        def outproj_mms(m, psy, k0, k1, start, stop):
                            for kt in range(k0, k1):
                                for ch in range(2):
                                    nc.tensor.matmul(
                                        psy[:, ch * 512:(ch + 1) * 512],
                                        wout_sb[:, kt,
                                                m * 128:(m + 1) * 128],
                                        ctxT_sb[:, kt,
                                                ch * 512:(ch + 1) * 512],
                                        start=(start and kt == k0),
                                        stop=(stop and kt == k1 - 1))

                        # m0/m1 accumulate their first three kt while the p3
                        # normalization + transposes drain; kt=3 joins after
                        psys = {}
                        for m in range(2):
                            psy = psSc.tile([128, 1024], F32,
                                            tag=f"sc{m % 2}", name=f"psy{m}")
                            psys[m] = psy
                            outproj_mms(m, psy, 0, 3, True, False)
                        emit_transpose(psQk, "qk", 3, 0)
                        emit_transpose(psDen, "den", 3, 1, ceng=nc.scalar)
                        for m in range(8):
                            if m < 2:
                                psy = psys[m]
                                outproj_mms(m, psy, 3, 4, False, True)
                            else:
                                psy = psSc.tile([128, 1024], F32,
                                                tag=f"sc{m % 2}",
                                                name=f"psy{m}")
                                outproj_mms(m, psy, 0, 4, True, True)
                            if m == 7:
                                # split the last tile across engines
                                # (separate tiles so the copies overlap)
                                ya = ytp.tile([128, 512], BF16, tag="ya",
                                              name="yt7a")
                                yb = ytp.tile([128, 512], BF16, tag="yb",
                                              name="yt7b")
                                nc.scalar.copy(ya[:], psy[:, 0:512])
                                nc.vector.tensor_copy(yb[:],
                                                      psy[:, 512:1024])
                                nc.sync.dma_start(
                                    yT[m * 128:(m + 1) * 128, 0:512], ya[:])
                                nc.gpsimd.dma_start(
                                    yT[m * 128:(m + 1) * 128, 512:1024],
                                    yb[:])
                                continue
                            yt = ytp.tile([128, 1024], BF16, tag="yt",
                                          name=f"yt{m}")
                            if m % 2 == 0:
                                nc.scalar.copy(yt[:], psy[:])
                            else:
                                nc.vector.tensor_copy(yt[:], psy[:])
                            deng = nc.sync if m % 2 == 0 else nc.gpsimd
                            deng.dma_start(yT[m * 128:(m + 1) * 128, :],
                                           yt[:])

    return nc


def _split_waits(nc, max_waits=1):
    """This walrus build rejects >1 sync-wait command per instruction; hoist
    extra waits onto preceding NoOps on the same engine/queue."""
    for bb in nc.main_func.blocks:
        new_insts = []
        for ins in bb.instructions:
            si = getattr(ins, "sync_info", None)
            if si is not None and si.on_wait and len(si.on_wait) > max_waits:
                waits = list(si.on_wait)
                head, rest = waits[:max_waits], waits[max_waits:]
                while rest:
                    chunk, rest = rest[:max_waits], rest[max_waits:]
                    new_insts.append(mybir.InstNoOp(
                        name=f"waitsplit-{nc.next_id()}", ins=[], outs=[],
                        sync_info=mybir.SyncInfo(on_wait=chunk, on_update=[]),
                        engine=ins.engine))
                ins.sync_info = mybir.SyncInfo(
                    on_wait=head, on_update=list(si.on_update or []))
            new_insts.append(ins)
        bb.instructions = new_insts


def make_core_inputs(x, attention_mask, Wqkv, bqkv, Wout):
    """Host-side shard prep: returns list of 8 in_maps (core = 2*b + g)."""
    BF = ml_dtypes.bfloat16
    Wr = np.ascontiguousarray(Wqkv).reshape(HID, 3, H, D)
    br = np.ascontiguousarray(bqkv).reshape(3, H, D)

    inv = 1.0 / (THETA ** (np.arange(0, D, 2, dtype=np.float64) / D))
    pos = np.arange(S, dtype=np.float64)
    freqs = pos[:, None] * inv[None, :]              # [S, 32]
    emb = np.concatenate([freqs, freqs], axis=1)     # [S, 64]
    cosT = np.cos(emb).T.astype(np.float32)          # [64, S]
    sgn = np.concatenate([-np.ones(32), np.ones(32)])[:, None]
    sinTs = (sgn * np.sin(emb).T).astype(np.float32)
    # QSC-scaled tables push the fp8 rope values out of e4m3 subnormals;
    # the exp scale divides the QSC^2 back out.
    cos2 = (np.concatenate([cosT, cosT], 0) * 8.0).astype(BF)   # [128, S]
    sin2 = (np.concatenate([sinTs, sinTs], 0) * 8.0).astype(BF)
    ident = np.eye(128, dtype=np.float32).astype(BF)
    pp = np.arange(128)
    shmap = (pp - pp % 64) + (pp % 64 + 32) % 64
    permTm = np.zeros((128, 128), dtype=np.float32)
    permTm[shmap, pp] = 1.0
    permTm = permTm.astype(BF)

    in_maps = []
    for c in range(NCORES):
        b, g = c // 2, c % 2
        hs = slice(g * HPC, (g + 1) * HPC)
        wqk = np.concatenate(
            [Wr[:, 0, hs, :].reshape(HID, 512),
             Wr[:, 1, hs, :].reshape(HID, 512)], axis=1)
        wv = Wr[:, 2, hs, :].reshape(HID, 512)
        bqk = np.concatenate(
            [br[0, hs].reshape(512), br[1, hs].reshape(512)]
        ).reshape(8, 128).T
        mcolv = attention_mask[b].astype(np.float32).reshape(ST, 128).T

        def tile128(a):  # [kt*128, c] -> [128, kt, c]
            return np.ascontiguousarray(
                a.reshape(-1, 128, a.shape[1]).transpose(1, 0, 2).astype(BF))

        in_maps.append({
            "xT": tile128(x[b].T),
            "wqk": tile128(wqk),
            "wv": tile128(wv),
            "wout": tile128(Wout[g * 512:(g + 1) * 512, :]),
            "bqk": np.ascontiguousarray(bqk.astype(np.float32)),
            "cosT": cos2, "sinT": sin2, "permT": permTm,
            "mcol": np.ascontiguousarray(mcolv),
            "mcolB": np.ascontiguousarray(mcolv.astype(BF)),
            "ident": ident,
        })
    return in_maps


_PROGRAM = None


def kernel(x, attention_mask, Wqkv, bqkv, Wout, bout, _trace=False):
    global _PROGRAM
    x = np.asarray(x)
    attention_mask = np.asarray(attention_mask)
    Wqkv = np.asarray(Wqkv)
    bqkv = np.asarray(bqkv)
    Wout = np.asarray(Wout)
    bout = np.asarray(bout)

    if _PROGRAM is None:
        _PROGRAM = build_program()
        _split_waits(_PROGRAM)
    nc = _PROGRAM

    in_maps = make_core_inputs(x, attention_mask, Wqkv, bqkv, Wout)
    res = run_bass_kernel_spmd(
        nc, in_maps, core_ids=list(range(NCORES)), trace=_trace)

    y = np.empty((B, S, HID), dtype=np.float32)
    for b in range(B):
        acc = (res.results[2 * b]["yT"].astype(np.float32)
               + res.results[2 * b + 1]["yT"].astype(np.float32))
        y[b] = acc.T
    # exact host-side bias corrections: v-bias shifts context by a constant
    # (attn rows sum to 1), q/k biases were applied on device.
    bv = bqkv[2 * HID:3 * HID].astype(np.float32)
    y += (bv @ Wout + bout).astype(np.float32)[None, None, :]
    if _trace:
        kernel.last_exec_time_ns = res.exec_time_ns
    return y
